# revision 19
# baseline (speedup 1.0000x reference)
"""Trainium2 Bass kernel for nn_Encoder_5171140624511.

2-layer LSTM encoder (B=256, T=1024, D_IN=256, H=512) + VAE latent head.
Sharding: data-parallel over batch across 8 NeuronCores (32 samples/core),
LSTM/projection weights replicated.

Layout strategy per core:
  - state h, c in [batch=32 partitions, H free]; z = x@Wx + h@Wh + b computed
    as [32, 4H] in PSUM with the *weights as the moving operand* (float32r,
    1 cycle/row at N=512) and hT/xT as the 128x32 stationary tiles.
  - h is re-transposed each step via 4 PE transposes into hT [128, 4, 32].
  - x is pre-transposed on the host to xT [2, 128, T, 32] so per-step
    stationary x tiles DMA straight in.
  - latent head (mean/sigma projections, reparameterization, loss partials)
    computed on device; host only concatenates shards and finishes the
    scalar loss reduction.
"""

import os
import sys

sys.path.insert(0, "/opt/trn_rl_repo")

import numpy as np

import concourse.bass as bass
import concourse.tile as tile
from concourse import bacc, mybir
from concourse.bass_utils import run_bass_kernel_spmd

B, T_FULL, D, H, L = 256, 1024, 256, 512, 128
FH = 4 * H  # 2048
NCORES = 8
BL = B // NCORES  # 32
KH = H // 128  # 4 k-tiles for H
KD = D // 128  # 2 k-tiles for D_IN
NCH = FH // 512  # 4 n-chunks of 512

F32 = mybir.dt.float32
F32R = mybir.dt.float32r
AF = mybir.ActivationFunctionType


def _r(ap):
    """View an AP as float32r for fast fp32 matmul."""
    return ap.bitcast(F32R)


def build_nc(T=T_FULL, S=8, use_f32r=True, T_data=None, timing_mode=False,
             unroll=False, decouple=False, strip_tr=False, strip_gates=False,
             use_bulk=False):
    """Build + compile the per-core Bass program. T must be divisible by S.

    T_data: DRAM extent of xT (defaults to T). A smaller T with full T_data
    gives a calibration kernel with identical I/O but less compute.
    """
    assert T % S == 0
    if T_data is None:
        T_data = S if timing_mode else T
    R = F32R if use_f32r else F32
    nc = bacc.Bacc(None, target_bir_lowering=False)

    xT_d = nc.dram_tensor("xT", [KD, 128, T_data, BL], R, kind="ExternalInput")
    eps_d = nc.dram_tensor("eps", [BL, L], F32, kind="ExternalInput")
    Wx1_d = nc.dram_tensor("Wx1", [KD, 128, FH], R, kind="ExternalInput")
    Wh1_d = nc.dram_tensor("Wh1", [KH, 128, FH], R, kind="ExternalInput")
    Wx2_d = nc.dram_tensor("Wx2", [KH, 128, FH], R, kind="ExternalInput")
    Wh2_d = nc.dram_tensor("Wh2", [KH, 128, FH], R, kind="ExternalInput")
    b1_d = nc.dram_tensor("b1", [1, FH], R, kind="ExternalInput")
    b2_d = nc.dram_tensor("b2", [1, FH], R, kind="ExternalInput")
    wms_d = nc.dram_tensor("wms", [KH, 128, 2 * L], R, kind="ExternalInput")
    bms_d = nc.dram_tensor("bms", [1, 2 * L], R, kind="ExternalInput")
    ones_d = nc.dram_tensor("ones", [1, 128], R, kind="ExternalInput")
    zst_d = nc.dram_tensor("zstate", [128, KH * BL], R, kind="ExternalInput")
    ident_d = nc.dram_tensor("ident", [128, 128], F32, kind="ExternalInput")

    ret_d = nc.dram_tensor("retval", [BL, L], F32, kind="ExternalOutput")
    lp_d = nc.dram_tensor("loss_part", [BL, 1], F32, kind="ExternalOutput")
    c2_d = nc.dram_tensor("c2_out", [BL, H], F32, kind="ExternalOutput")
    h1_d = nc.dram_tensor("h1_out", [BL, H], F32, kind="ExternalOutput")
    c1_d = nc.dram_tensor("c1_out", [BL, H], F32, kind="ExternalOutput")

    with tile.TileContext(nc) as tc:
        with (
            tc.tile_pool(name="wpool", bufs=1) as wp,
            tc.tile_pool(name="xpool", bufs=2) as xp,
            tc.tile_pool(name="zxpool", bufs=1) as zxp,
            tc.tile_pool(name="state", bufs=1) as sp,
            tc.tile_pool(name="gates", bufs=1) as gp,
            tc.tile_pool(name="pz1", bufs=4, space="PSUM") as pz1,
            tc.tile_pool(name="pz2", bufs=4, space="PSUM") as pz2,
            tc.tile_pool(name="opool", bufs=1) as op,
        ):
            # ---- load weights & constants ----
            Wx1 = wp.tile([128, KD, FH], R, tag="Wx1")
            Wh1 = wp.tile([128, KH, FH], R, tag="Wh1")
            Wx2 = wp.tile([128, KH, FH], R, tag="Wx2")
            Wh2 = wp.tile([128, KH, FH], R, tag="Wh2")
            wms = wp.tile([128, KH, 2 * L], R, tag="wms")
            b1 = wp.tile([1, FH], R, tag="b1")
            b2 = wp.tile([1, FH], R, tag="b2")
            bms = wp.tile([1, 2 * L], R, tag="bms")
            ones = wp.tile([1, 128], R, tag="ones")
            ident = wp.tile([128, 128], F32, tag="ident")
            eps = wp.tile([BL, L], F32, tag="eps")

            nc.sync.dma_start(Wx1[:], Wx1_d.ap().rearrange("k p n -> p k n"))
            nc.sync.dma_start(Wh1[:], Wh1_d.ap().rearrange("k p n -> p k n"))
            nc.sync.dma_start(Wx2[:], Wx2_d.ap().rearrange("k p n -> p k n"))
            nc.sync.dma_start(Wh2[:], Wh2_d.ap().rearrange("k p n -> p k n"))
            nc.sync.dma_start(wms[:], wms_d.ap().rearrange("k p n -> p k n"))
            nc.sync.dma_start(b1[:], b1_d[:])
            nc.sync.dma_start(b2[:], b2_d[:])
            nc.sync.dma_start(bms[:], bms_d[:])
            nc.sync.dma_start(ones[:], ones_d[:])
            nc.sync.dma_start(ident[:], ident_d[:])
            nc.sync.dma_start(eps[:], eps_d[:])

            # ---- persistent state ----
            h1 = sp.tile([BL, H], F32, tag="h1")
            c1 = sp.tile([BL, H], F32, tag="c1")
            h2 = sp.tile([BL, H], F32, tag="h2")
            c2 = sp.tile([BL, H], F32, tag="c2")
            h1T = sp.tile([128, KH, BL], R, tag="h1T")
            h2T = sp.tile([128, KH, BL], R, tag="h2T")
            for t_ in (h1, c1, h2, c2):
                nc.vector.memset(t_[:], 0.0)
            nc.sync.dma_start(h1T.rearrange("p k b -> p (k b)"), zst_d[:])
            nc.sync.dma_start(h2T.rearrange("p k b -> p (k b)"), zst_d[:])
            if decouple:
                # timing experiment: MMs read frozen copies of hT so the
                # cross-step dependency chain is cut; instruction mix identical
                h1Tc = sp.tile([128, KH, BL], R, tag="h1Tc")
                h2Tc = sp.tile([128, KH, BL], R, tag="h2Tc")
                nc.sync.dma_start(h1Tc.rearrange("p k b -> p (k b)"), zst_d[:])
                nc.sync.dma_start(h2Tc.rearrange("p k b -> p (k b)"), zst_d[:])
            else:
                h1Tc, h2Tc = h1T, h2T

            def transpose_to(hsrc, hTdst, force=False):
                """hsrc [BL, H] -> hTdst [128, KH, BL] via PE transposes."""
                if strip_tr and not force:
                    return
                ps = pz1.tile([128, KH * BL], F32, tag="z1")
                for k in range(KH):
                    nc.tensor.transpose(
                        ps[:, k * BL : (k + 1) * BL],
                        hsrc[:, k * 128 : (k + 1) * 128],
                        ident[:BL, :BL],
                    )
                nc.vector.tensor_copy(hTdst.rearrange("p k b -> p (k b)"), ps[:])

            def lstm_step(*a, **kw):
                raise NotImplementedError

            # bias broadcast tiles [BL, FH] (one-time build via rank-1 MMs)
            b1b = wp.tile([BL, FH], F32, tag="b1b")
            b2b = wp.tile([BL, FH], F32, tag="b2b")
            for bsrc, bdst in ((b1, b1b), (b2, b2b)):
                for n in range(NCH):
                    nsl = slice(n * 512, (n + 1) * 512)
                    pt = pz2.tile([BL, 512], F32, tag="z2")
                    nc.tensor.matmul(
                        pt[:], ones[:1, :BL], bsrc[:1, nsl],
                        start=True, stop=True,
                    )
                    nc.vector.tensor_copy(bdst[:, nsl], pt[:])

            def gates_and_update(zps, adds, h, c, ztag):
                if strip_gates:
                    return
                g = {}
                for n in range(NCH):
                    nsl = slice(n * 512, (n + 1) * 512)
                    for a_ in adds:
                        nc.vector.tensor_add(zps[n][:], zps[n][:], a_[:, nsl])
                    gt = gp.tile([BL, 512], F32, tag=f"{ztag}g{n}")
                    fn = AF.Tanh if n == 2 else AF.Sigmoid
                    nc.scalar.activation(gt[:], zps[n][:], fn)
                    g[n] = gt
                i_, f_, g_, o_ = g[0], g[1], g[2], g[3]
                t1 = gp.tile([BL, H], F32, tag="t1")
                t2 = gp.tile([BL, H], F32, tag="t2")
                nc.vector.tensor_mul(t1[:], f_[:], c[:])
                nc.vector.tensor_mul(t2[:], i_[:], g_[:])
                nc.vector.tensor_add(c[:], t1[:], t2[:])
                th = gp.tile([BL, H], F32, tag="th")
                nc.scalar.activation(th[:], c[:], AF.Tanh)
                nc.vector.tensor_mul(h[:], o_[:], th[:])

            def emit_step(z1adds, carry, x_stat=None):
                """Emit one LSTM step; layer-2 gate/transpose of the previous
                step is deferred into this step's z1 window (carry holds the
                previous step's z2 PSUM chunks). z1adds: tensors DVE-added to
                the layer-1 PSUM. x_stat: per-step x stationary (non-bulk)."""
                z1ps = []
                for n in range(NCH):
                    nsl = slice(n * 512, (n + 1) * 512)
                    zp = pz1.tile([BL, 512], F32, tag="z1")
                    if x_stat is not None:
                        for k in range(KD):
                            nc.tensor.matmul(
                                zp[:], x_stat(k), Wx1[:, k, nsl],
                                start=(k == 0), stop=False,
                            )
                    for k in range(KH):
                        nc.tensor.matmul(
                            zp[:], h1Tc[:, k, :], Wh1[:, k, nsl],
                            start=(x_stat is None and k == 0),
                            stop=(k == KH - 1),
                        )
                    z1ps.append(zp)
                # finish the PREVIOUS step's layer 2 while z1 MMs run
                if carry is not None:
                    gates_and_update(carry, [b2b], h2, c2, "z2")
                    transpose_to(h2, h2T)
                # layer-2 z, recurrent half (h2T now current)
                z2ps = []
                for n in range(NCH):
                    nsl = slice(n * 512, (n + 1) * 512)
                    zp = pz2.tile([BL, 512], F32, tag="z2")
                    for k in range(KH):
                        nc.tensor.matmul(
                            zp[:], h2Tc[:, k, :], Wh2[:, k, nsl],
                            start=(k == 0), stop=False,
                        )
                    z2ps.append(zp)
                gates_and_update(z1ps, z1adds, h1, c1, "z1")
                transpose_to(h1, h1T)
                # layer-2 z, input half (needs fresh h1T)
                for n in range(NCH):
                    nsl = slice(n * 512, (n + 1) * 512)
                    for k in range(KH):
                        nc.tensor.matmul(
                            z2ps[n][:], h1Tc[:, k, :], Wx2[:, k, nsl],
                            start=False, stop=(k == KH - 1),
                        )
                return z2ps

            # ---- time loop ----
            def chunk_body(iv):
                xt = xp.tile([128, KD, S, BL], R, tag="xT")
                xsl = slice(0, S) if timing_mode else bass.ds(iv, S)
                nc.sync.dma_start(
                    xt[:],
                    xT_d.ap()[:, :, xsl, :].rearrange(
                        "k p t b -> p k t b"
                    ),
                )
                carry = None
                if use_bulk:
                    # bulk: zx = x@Wx1 for the whole chunk at M=128 (full PE
                    # util). NOTE: measured slower than per-step x MMs — the
                    # PSUM->SBUF copies delay gate activations. Kept for
                    # experiments.
                    JT = S // 4
                    zxa = zxp.tile([128, JT, FH], F32, tag="zx")
                    for j in range(JT):
                        for n in range(NCH):
                            nsl = slice(n * 512, (n + 1) * 512)
                            bp = pz1.tile([128, 512], F32, tag="z1")
                            for k in range(KD):
                                nc.tensor.matmul(
                                    bp[:],
                                    xt[:, k, 4 * j : 4 * j + 4, :].rearrange(
                                        "p t b -> p (t b)"
                                    ),
                                    Wx1[:, k, nsl],
                                    start=(k == 0), stop=(k == KD - 1),
                                )
                            nc.scalar.copy(zxa[:, j, nsl], bp[:])
                    for s in range(S):
                        zxs = zxa[32 * (s % 4) : 32 * (s % 4) + 32, s // 4, :]
                        carry = emit_step([b1b, zxs], carry)
                else:
                    for s in range(S):
                        carry = emit_step(
                            [b1b], carry, x_stat=lambda k, s=s: xt[:, k, s, :]
                        )
                # drain the last step's layer 2 before the back-edge barrier
                gates_and_update(carry, [b2b], h2, c2, "z2")
                transpose_to(h2, h2T)

            if unroll:
                assert timing_mode or T == S
                for it in range(T // S):
                    chunk_body(it * S)
            elif T // S > 1:
                with tc.For_i(0, T, S, hint_engines=(mybir.EngineType.PE,)) as iv:
                    chunk_body(iv)
            else:
                chunk_body(0)

            # ---- latent head ----
            c2T = sp.tile([128, KH, BL], R, tag="c2T")
            transpose_to(c2, c2T, force=True)
            pms = pz1.tile([BL, 512], F32, tag="z1")
            for k in range(KH):
                nc.tensor.matmul(
                    pms[:, : 2 * L], c2T[:, k, :], wms[:, k, :],
                    start=(k == 0), stop=False,
                )
            nc.tensor.matmul(
                pms[:, : 2 * L], ones[:1, :BL], bms[:1, :],
                start=False, stop=True,
            )
            mean = pms[:, 0:L]
            sigma = pms[:, L : 2 * L]

            ehalf = op.tile([BL, L], F32, tag="ehalf")
            nc.scalar.activation(ehalf[:], sigma, AF.Exp, scale=0.5)
            rv = op.tile([BL, L], F32, tag="rv")
            nc.vector.tensor_mul(rv[:], ehalf[:], eps[:])
            nc.vector.tensor_add(rv[:], rv[:], mean)
            nc.sync.dma_start(ret_d[:], rv[:])

            # loss partials: u = sigma - mean^2 - exp(sigma), summed over L
            sq = op.tile([BL, L], F32, tag="sq")
            nc.scalar.activation(sq[:], mean, AF.Square)
            ex = op.tile([BL, L], F32, tag="ex")
            nc.scalar.activation(ex[:], sigma, AF.Exp)
            u = op.tile([BL, L], F32, tag="u")
            nc.vector.tensor_sub(u[:], sigma, sq[:])
            nc.vector.tensor_sub(u[:], u[:], ex[:])
            lp = op.tile([BL, 1], F32, tag="lp")
            nc.vector.reduce_sum(lp[:], u[:], axis=mybir.AxisListType.X)
            nc.sync.dma_start(lp_d[:], lp[:])

            co = op.tile([BL, H], F32, tag="co")
            nc.vector.tensor_copy(co[:], c2[:])
            nc.sync.dma_start(c2_d[:], co[:])
            ho = op.tile([BL, H], F32, tag="co")
            nc.vector.tensor_copy(ho[:], h1[:])
            nc.sync.dma_start(h1_d[:], ho[:])
            c1o = op.tile([BL, H], F32, tag="co")
            nc.vector.tensor_copy(c1o[:], c1[:])
            nc.sync.dma_start(c1_d[:], c1o[:])

    nc.compile()
    return nc


def prep_core_inputs(inputs, eps, Wx1, Wh1, b1, Wx2, Wh2, b2,
                     w_mean, b_mean, w_sigma, b_sigma, T=T_FULL):
    """Build the per-core in_maps list (host-side shard + relayout)."""
    f = np.float32
    shared = {
        "Wx1": np.ascontiguousarray(Wx1.reshape(KD, 128, FH), f),
        "Wh1": np.ascontiguousarray(Wh1.reshape(KH, 128, FH), f),
        "Wx2": np.ascontiguousarray(Wx2.reshape(KH, 128, FH), f),
        "Wh2": np.ascontiguousarray(Wh2.reshape(KH, 128, FH), f),
        "b1": np.ascontiguousarray(b1.reshape(1, FH), f),
        "b2": np.ascontiguousarray(b2.reshape(1, FH), f),
        "wms": np.ascontiguousarray(
            np.concatenate([w_mean, w_sigma], axis=1).reshape(KH, 128, 2 * L), f
        ),
        "bms": np.ascontiguousarray(
            np.concatenate([b_mean, b_sigma]).reshape(1, 2 * L), f
        ),
        "ones": np.ones((1, 128), f),
        "zstate": np.zeros((128, KH * BL), f),
        "ident": np.eye(128, dtype=f),
    }
    in_maps = []
    for c in range(NCORES):
        sl = slice(c * BL, (c + 1) * BL)
        xc = np.asarray(inputs[sl, :T], f)  # [BL, T, D]
        xT = np.ascontiguousarray(xc.transpose(2, 1, 0)).reshape(KD, 128, T, BL)
        m = dict(shared)
        m["xT"] = xT
        m["eps"] = np.ascontiguousarray(eps[sl], f)
        in_maps.append(m)
    return in_maps


_NC_CACHE = {}


def _get_nc(T=T_FULL, S=8):
    key = (T, S)
    if key not in _NC_CACHE:
        _NC_CACHE[key] = build_nc(T=T, S=S)
    return _NC_CACHE[key]


def kernel(inputs, eps, Wx1, Wh1, b1, Wx2, Wh2, b2,
           w_mean, b_mean, w_sigma, b_sigma):
    T = inputs.shape[1]
    nc = _get_nc(T=T, S=8)
    in_maps = prep_core_inputs(
        inputs, eps, Wx1, Wh1, b1, Wx2, Wh2, b2,
        w_mean, b_mean, w_sigma, b_sigma, T=T,
    )
    res = run_bass_kernel_spmd(nc, in_maps, core_ids=list(range(NCORES)))
    retval = np.concatenate([r["retval"] for r in res.results], axis=0)
    total = float(sum(r["loss_part"].sum() for r in res.results))
    loss = np.float32(-0.5 * (1.0 + total / (B * L)))
    return retval, loss


# revision 20
# speedup vs baseline: 1.0723x; 1.0723x over previous
"""Trainium2 Bass kernel for nn_Encoder_5171140624511.

2-layer LSTM encoder (B=256, T=1024, D_IN=256, H=512) + VAE latent head.
Sharding: data-parallel over batch across 8 NeuronCores (32 samples/core),
LSTM/projection weights replicated.

Layout strategy per core:
  - state h, c in [batch=32 partitions, H free]; z = x@Wx + h@Wh + b computed
    as [32, 4H] in PSUM with the *weights as the moving operand* (float32r,
    1 cycle/row at N=512) and hT/xT as the 128x32 stationary tiles.
  - h is re-transposed each step via 4 PE transposes into hT [128, 4, 32].
  - x is pre-transposed on the host to xT [2, 128, T, 32] so per-step
    stationary x tiles DMA straight in.
  - latent head (mean/sigma projections, reparameterization, loss partials)
    computed on device; host only concatenates shards and finishes the
    scalar loss reduction.
"""

import os
import sys

sys.path.insert(0, "/opt/trn_rl_repo")

import numpy as np

import concourse.bass as bass
import concourse.tile as tile
from concourse import bacc, mybir
from concourse.bass_utils import run_bass_kernel_spmd

B, T_FULL, D, H, L = 256, 1024, 256, 512, 128
FH = 4 * H  # 2048
NCORES = 8
BL = B // NCORES  # 32
KH = H // 128  # 4 k-tiles for H
KD = D // 128  # 2 k-tiles for D_IN
NCH = FH // 512  # 4 n-chunks of 512

F32 = mybir.dt.float32
F32R = mybir.dt.float32r
AF = mybir.ActivationFunctionType


def _r(ap):
    """View an AP as float32r for fast fp32 matmul."""
    return ap.bitcast(F32R)


def build_nc(T=T_FULL, S=8, use_f32r=True, T_data=None, timing_mode=False,
             unroll=False, decouple=False, strip_tr=False, strip_gates=False,
             use_bulk=False):
    """Build + compile the per-core Bass program. T must be divisible by S.

    T_data: DRAM extent of xT (defaults to T). A smaller T with full T_data
    gives a calibration kernel with identical I/O but less compute.
    """
    assert T % S == 0
    if T_data is None:
        T_data = S if timing_mode else T
    R = F32R if use_f32r else F32
    nc = bacc.Bacc(None, target_bir_lowering=False)

    xT_d = nc.dram_tensor("xT", [KD, 128, T_data, BL], R, kind="ExternalInput")
    eps_d = nc.dram_tensor("eps", [BL, L], F32, kind="ExternalInput")
    Wx1_d = nc.dram_tensor("Wx1", [KD, 128, FH], R, kind="ExternalInput")
    Wh1_d = nc.dram_tensor("Wh1", [KH, 128, FH], R, kind="ExternalInput")
    Wx2_d = nc.dram_tensor("Wx2", [KH, 128, FH], R, kind="ExternalInput")
    Wh2_d = nc.dram_tensor("Wh2", [KH, 128, FH], R, kind="ExternalInput")
    b1_d = nc.dram_tensor("b1", [1, FH], R, kind="ExternalInput")
    b2_d = nc.dram_tensor("b2", [1, FH], R, kind="ExternalInput")
    wms_d = nc.dram_tensor("wms", [KH, 128, 2 * L], R, kind="ExternalInput")
    bms_d = nc.dram_tensor("bms", [1, 2 * L], R, kind="ExternalInput")
    ones_d = nc.dram_tensor("ones", [1, 128], R, kind="ExternalInput")
    zst_d = nc.dram_tensor("zstate", [128, KH * BL], R, kind="ExternalInput")
    ident_d = nc.dram_tensor("ident", [128, 128], F32, kind="ExternalInput")

    ret_d = nc.dram_tensor("retval", [BL, L], F32, kind="ExternalOutput")
    lp_d = nc.dram_tensor("loss_part", [BL, 1], F32, kind="ExternalOutput")
    c2_d = nc.dram_tensor("c2_out", [BL, H], F32, kind="ExternalOutput")
    h1_d = nc.dram_tensor("h1_out", [BL, H], F32, kind="ExternalOutput")
    c1_d = nc.dram_tensor("c1_out", [BL, H], F32, kind="ExternalOutput")

    with tile.TileContext(nc) as tc:
        with (
            tc.tile_pool(name="wpool", bufs=1) as wp,
            tc.tile_pool(name="xpool", bufs=2) as xp,
            tc.tile_pool(name="zxpool", bufs=1) as zxp,
            tc.tile_pool(name="state", bufs=1) as sp,
            tc.tile_pool(name="gates", bufs=1) as gp,
            tc.tile_pool(name="pz1", bufs=4, space="PSUM") as pz1,
            tc.tile_pool(name="pz2", bufs=4, space="PSUM") as pz2,
            tc.tile_pool(name="opool", bufs=1) as op,
        ):
            # ---- load weights & constants ----
            Wx1 = wp.tile([128, KD, FH], R, tag="Wx1")
            Wh1 = wp.tile([128, KH, FH], R, tag="Wh1")
            Wx2 = wp.tile([128, KH, FH], R, tag="Wx2")
            Wh2 = wp.tile([128, KH, FH], R, tag="Wh2")
            wms = wp.tile([128, KH, 2 * L], R, tag="wms")
            b1 = wp.tile([1, FH], R, tag="b1")
            b2 = wp.tile([1, FH], R, tag="b2")
            bms = wp.tile([1, 2 * L], R, tag="bms")
            ones = wp.tile([1, 128], R, tag="ones")
            ident = wp.tile([128, 128], F32, tag="ident")
            eps = wp.tile([BL, L], F32, tag="eps")

            nc.sync.dma_start(Wx1[:], Wx1_d.ap().rearrange("k p n -> p k n"))
            nc.sync.dma_start(Wh1[:], Wh1_d.ap().rearrange("k p n -> p k n"))
            nc.sync.dma_start(Wx2[:], Wx2_d.ap().rearrange("k p n -> p k n"))
            nc.sync.dma_start(Wh2[:], Wh2_d.ap().rearrange("k p n -> p k n"))
            nc.sync.dma_start(wms[:], wms_d.ap().rearrange("k p n -> p k n"))
            nc.sync.dma_start(b1[:], b1_d[:])
            nc.sync.dma_start(b2[:], b2_d[:])
            nc.sync.dma_start(bms[:], bms_d[:])
            nc.sync.dma_start(ones[:], ones_d[:])
            nc.sync.dma_start(ident[:], ident_d[:])
            nc.sync.dma_start(eps[:], eps_d[:])

            # ---- persistent state ----
            h1 = sp.tile([BL, H], F32, tag="h1")
            c1 = sp.tile([BL, H], F32, tag="c1")
            h2 = sp.tile([BL, H], F32, tag="h2")
            c2 = sp.tile([BL, H], F32, tag="c2")
            h1T = sp.tile([128, KH, BL], R, tag="h1T")
            h2T = sp.tile([128, KH, BL], R, tag="h2T")
            for t_ in (h1, c1, h2, c2):
                nc.vector.memset(t_[:], 0.0)
            nc.sync.dma_start(h1T.rearrange("p k b -> p (k b)"), zst_d[:])
            nc.sync.dma_start(h2T.rearrange("p k b -> p (k b)"), zst_d[:])
            if decouple:
                # timing experiment: MMs read frozen copies of hT so the
                # cross-step dependency chain is cut; instruction mix identical
                h1Tc = sp.tile([128, KH, BL], R, tag="h1Tc")
                h2Tc = sp.tile([128, KH, BL], R, tag="h2Tc")
                nc.sync.dma_start(h1Tc.rearrange("p k b -> p (k b)"), zst_d[:])
                nc.sync.dma_start(h2Tc.rearrange("p k b -> p (k b)"), zst_d[:])
            else:
                h1Tc, h2Tc = h1T, h2T

            def transpose_to(hsrc, hTdst, force=False):
                """hsrc [BL, H] -> hTdst [128, KH, BL] via PE transposes."""
                if strip_tr and not force:
                    return
                ps = pz1.tile([128, KH * BL], F32, tag="z1")
                for k in range(KH):
                    nc.tensor.transpose(
                        ps[:, k * BL : (k + 1) * BL],
                        hsrc[:, k * 128 : (k + 1) * 128],
                        ident[:BL, :BL],
                    )
                nc.vector.tensor_copy(hTdst.rearrange("p k b -> p (k b)"), ps[:])

            def lstm_step(*a, **kw):
                raise NotImplementedError

            # bias broadcast tiles [BL, FH] (one-time build via rank-1 MMs)
            b1b = wp.tile([BL, FH], F32, tag="b1b")
            b2b = wp.tile([BL, FH], F32, tag="b2b")
            for bsrc, bdst in ((b1, b1b), (b2, b2b)):
                for n in range(NCH):
                    nsl = slice(n * 512, (n + 1) * 512)
                    pt = pz2.tile([BL, 512], F32, tag="z2")
                    nc.tensor.matmul(
                        pt[:], ones[:1, :BL], bsrc[:1, nsl],
                        start=True, stop=True,
                    )
                    nc.vector.tensor_copy(bdst[:, nsl], pt[:])

            def gates_and_update(zps, adds, h, c, ztag):
                if strip_gates:
                    return
                g = {}
                for n in range(NCH):
                    nsl = slice(n * 512, (n + 1) * 512)
                    for a_ in adds:
                        nc.vector.tensor_add(zps[n][:], zps[n][:], a_[:, nsl])
                    gt = gp.tile([BL, 512], F32, tag=f"{ztag}g{n}")
                    fn = AF.Tanh if n == 2 else AF.Sigmoid
                    nc.scalar.activation(gt[:], zps[n][:], fn)
                    g[n] = gt
                i_, f_, g_, o_ = g[0], g[1], g[2], g[3]
                t1 = gp.tile([BL, H], F32, tag="t1")
                t2 = gp.tile([BL, H], F32, tag="t2")
                nc.vector.tensor_mul(t1[:], f_[:], c[:])
                nc.vector.tensor_mul(t2[:], i_[:], g_[:])
                nc.vector.tensor_add(c[:], t1[:], t2[:])
                th = gp.tile([BL, H], F32, tag="th")
                nc.scalar.activation(th[:], c[:], AF.Tanh)
                nc.vector.tensor_mul(h[:], o_[:], th[:])

            def emit_step(z1adds, carry, x_stat=None):
                """Emit one LSTM step; layer-2 gate/transpose of the previous
                step is deferred into this step's z1 window (carry holds the
                previous step's z2 PSUM chunks). z1adds: tensors DVE-added to
                the layer-1 PSUM. x_stat: per-step x stationary (non-bulk)."""
                z1ps = []
                for n in range(NCH):
                    nsl = slice(n * 512, (n + 1) * 512)
                    zp = pz1.tile([BL, 512], F32, tag="z1")
                    if x_stat is not None:
                        for k in range(KD):
                            nc.tensor.matmul(
                                zp[:], x_stat(k), Wx1[:, k, nsl],
                                start=(k == 0), stop=False,
                            )
                    for k in range(KH):
                        nc.tensor.matmul(
                            zp[:], h1Tc[:, k, :], Wh1[:, k, nsl],
                            start=(x_stat is None and k == 0),
                            stop=(k == KH - 1),
                        )
                    z1ps.append(zp)
                # finish the PREVIOUS step's layer 2 while z1 MMs run
                if carry is not None:
                    gates_and_update(carry, [b2b], h2, c2, "z2")
                    transpose_to(h2, h2T)
                # layer-2 z, recurrent half (h2T now current)
                z2ps = []
                for n in range(NCH):
                    nsl = slice(n * 512, (n + 1) * 512)
                    zp = pz2.tile([BL, 512], F32, tag="z2")
                    for k in range(KH):
                        nc.tensor.matmul(
                            zp[:], h2Tc[:, k, :], Wh2[:, k, nsl],
                            start=(k == 0), stop=False,
                        )
                    z2ps.append(zp)
                gates_and_update(z1ps, z1adds, h1, c1, "z1")
                transpose_to(h1, h1T)
                # layer-2 z, input half (needs fresh h1T)
                for n in range(NCH):
                    nsl = slice(n * 512, (n + 1) * 512)
                    for k in range(KH):
                        nc.tensor.matmul(
                            z2ps[n][:], h1Tc[:, k, :], Wx2[:, k, nsl],
                            start=False, stop=(k == KH - 1),
                        )
                return z2ps

            # ---- time loop ----
            def chunk_body(iv):
                xt = xp.tile([128, KD, S, BL], R, tag="xT")
                xsl = slice(0, S) if timing_mode else bass.ds(iv, S)
                nc.sync.dma_start(
                    xt[:],
                    xT_d.ap()[:, :, xsl, :].rearrange(
                        "k p t b -> p k t b"
                    ),
                )
                carry = None
                if use_bulk:
                    # bulk: zx = x@Wx1 for the whole chunk at M=128 (full PE
                    # util). NOTE: measured slower than per-step x MMs — the
                    # PSUM->SBUF copies delay gate activations. Kept for
                    # experiments.
                    JT = S // 4
                    zxa = zxp.tile([128, JT, FH], F32, tag="zx")
                    for j in range(JT):
                        for n in range(NCH):
                            nsl = slice(n * 512, (n + 1) * 512)
                            bp = pz1.tile([128, 512], F32, tag="z1")
                            for k in range(KD):
                                nc.tensor.matmul(
                                    bp[:],
                                    xt[:, k, 4 * j : 4 * j + 4, :].rearrange(
                                        "p t b -> p (t b)"
                                    ),
                                    Wx1[:, k, nsl],
                                    start=(k == 0), stop=(k == KD - 1),
                                )
                            nc.scalar.copy(zxa[:, j, nsl], bp[:])
                    for s in range(S):
                        zxs = zxa[32 * (s % 4) : 32 * (s % 4) + 32, s // 4, :]
                        carry = emit_step([b1b, zxs], carry)
                else:
                    for s in range(S):
                        carry = emit_step(
                            [b1b], carry, x_stat=lambda k, s=s: xt[:, k, s, :]
                        )
                # drain the last step's layer 2 before the back-edge barrier
                gates_and_update(carry, [b2b], h2, c2, "z2")
                transpose_to(h2, h2T)

            if unroll:
                assert timing_mode or T == S
                for it in range(T // S):
                    chunk_body(it * S)
            elif T // S > 1:
                with tc.For_i(0, T, S, hint_engines=(mybir.EngineType.PE,)) as iv:
                    chunk_body(iv)
            else:
                chunk_body(0)

            # ---- latent head ----
            c2T = sp.tile([128, KH, BL], R, tag="c2T")
            transpose_to(c2, c2T, force=True)
            pms = pz1.tile([BL, 512], F32, tag="z1")
            for k in range(KH):
                nc.tensor.matmul(
                    pms[:, : 2 * L], c2T[:, k, :], wms[:, k, :],
                    start=(k == 0), stop=False,
                )
            nc.tensor.matmul(
                pms[:, : 2 * L], ones[:1, :BL], bms[:1, :],
                start=False, stop=True,
            )
            mean = pms[:, 0:L]
            sigma = pms[:, L : 2 * L]

            ehalf = op.tile([BL, L], F32, tag="ehalf")
            nc.scalar.activation(ehalf[:], sigma, AF.Exp, scale=0.5)
            rv = op.tile([BL, L], F32, tag="rv")
            nc.vector.tensor_mul(rv[:], ehalf[:], eps[:])
            nc.vector.tensor_add(rv[:], rv[:], mean)
            nc.sync.dma_start(ret_d[:], rv[:])

            # loss partials: u = sigma - mean^2 - exp(sigma), summed over L
            sq = op.tile([BL, L], F32, tag="sq")
            nc.scalar.activation(sq[:], mean, AF.Square)
            ex = op.tile([BL, L], F32, tag="ex")
            nc.scalar.activation(ex[:], sigma, AF.Exp)
            u = op.tile([BL, L], F32, tag="u")
            nc.vector.tensor_sub(u[:], sigma, sq[:])
            nc.vector.tensor_sub(u[:], u[:], ex[:])
            lp = op.tile([BL, 1], F32, tag="lp")
            nc.vector.reduce_sum(lp[:], u[:], axis=mybir.AxisListType.X)
            nc.sync.dma_start(lp_d[:], lp[:])

            co = op.tile([BL, H], F32, tag="co")
            nc.vector.tensor_copy(co[:], c2[:])
            nc.sync.dma_start(c2_d[:], co[:])
            ho = op.tile([BL, H], F32, tag="co")
            nc.vector.tensor_copy(ho[:], h1[:])
            nc.sync.dma_start(h1_d[:], ho[:])
            c1o = op.tile([BL, H], F32, tag="co")
            nc.vector.tensor_copy(c1o[:], c1[:])
            nc.sync.dma_start(c1_d[:], c1o[:])

    nc.compile()
    return nc


def prep_core_inputs(inputs, eps, Wx1, Wh1, b1, Wx2, Wh2, b2,
                     w_mean, b_mean, w_sigma, b_sigma, T=T_FULL):
    """Build the per-core in_maps list (host-side shard + relayout)."""
    f = np.float32
    shared = {
        "Wx1": np.ascontiguousarray(Wx1.reshape(KD, 128, FH), f),
        "Wh1": np.ascontiguousarray(Wh1.reshape(KH, 128, FH), f),
        "Wx2": np.ascontiguousarray(Wx2.reshape(KH, 128, FH), f),
        "Wh2": np.ascontiguousarray(Wh2.reshape(KH, 128, FH), f),
        "b1": np.ascontiguousarray(b1.reshape(1, FH), f),
        "b2": np.ascontiguousarray(b2.reshape(1, FH), f),
        "wms": np.ascontiguousarray(
            np.concatenate([w_mean, w_sigma], axis=1).reshape(KH, 128, 2 * L), f
        ),
        "bms": np.ascontiguousarray(
            np.concatenate([b_mean, b_sigma]).reshape(1, 2 * L), f
        ),
        "ones": np.ones((1, 128), f),
        "zstate": np.zeros((128, KH * BL), f),
        "ident": np.eye(128, dtype=f),
    }
    in_maps = []
    for c in range(NCORES):
        sl = slice(c * BL, (c + 1) * BL)
        xc = np.asarray(inputs[sl, :T], f)  # [BL, T, D]
        xT = np.ascontiguousarray(xc.transpose(2, 1, 0)).reshape(KD, 128, T, BL)
        m = dict(shared)
        m["xT"] = xT
        m["eps"] = np.ascontiguousarray(eps[sl], f)
        in_maps.append(m)
    return in_maps


_NC_CACHE = {}


def _get_nc(T=T_FULL, S=16):
    key = (T, S)
    if key not in _NC_CACHE:
        _NC_CACHE[key] = build_nc(T=T, S=S)
    return _NC_CACHE[key]


def kernel(inputs, eps, Wx1, Wh1, b1, Wx2, Wh2, b2,
           w_mean, b_mean, w_sigma, b_sigma):
    T = inputs.shape[1]
    nc = _get_nc(T=T, S=16)
    in_maps = prep_core_inputs(
        inputs, eps, Wx1, Wh1, b1, Wx2, Wh2, b2,
        w_mean, b_mean, w_sigma, b_sigma, T=T,
    )
    res = run_bass_kernel_spmd(nc, in_maps, core_ids=list(range(NCORES)))
    retval = np.concatenate([r["retval"] for r in res.results], axis=0)
    total = float(sum(r["loss_part"].sum() for r in res.results))
    loss = np.float32(-0.5 * (1.0 + total / (B * L)))
    return retval, loss


# revision 21
# speedup vs baseline: 1.1050x; 1.0305x over previous
"""Trainium2 Bass kernel for nn_Encoder_5171140624511.

2-layer LSTM encoder (B=256, T=1024, D_IN=256, H=512) + VAE latent head.
Sharding: data-parallel over batch across 8 NeuronCores (32 samples/core),
LSTM/projection weights replicated.

Layout strategy per core:
  - state h, c in [batch=32 partitions, H free]; z = x@Wx + h@Wh + b computed
    as [32, 4H] in PSUM with the *weights as the moving operand* (float32r,
    1 cycle/row at N=512) and hT/xT as the 128x32 stationary tiles.
  - h is re-transposed each step via 4 PE transposes into hT [128, 4, 32].
  - x is pre-transposed on the host to xT [2, 128, T, 32] so per-step
    stationary x tiles DMA straight in.
  - latent head (mean/sigma projections, reparameterization, loss partials)
    computed on device; host only concatenates shards and finishes the
    scalar loss reduction.
"""

import os
import sys

sys.path.insert(0, "/opt/trn_rl_repo")

import numpy as np

import concourse.bass as bass
import concourse.tile as tile
from concourse import bacc, mybir
from concourse.bass_utils import run_bass_kernel_spmd

B, T_FULL, D, H, L = 256, 1024, 256, 512, 128
FH = 4 * H  # 2048
NCORES = 8
BL = B // NCORES  # 32
KH = H // 128  # 4 k-tiles for H
KD = D // 128  # 2 k-tiles for D_IN
NCH = FH // 512  # 4 n-chunks of 512

F32 = mybir.dt.float32
F32R = mybir.dt.float32r
AF = mybir.ActivationFunctionType


def _r(ap):
    """View an AP as float32r for fast fp32 matmul."""
    return ap.bitcast(F32R)


def build_nc(T=T_FULL, S=8, use_f32r=True, T_data=None, timing_mode=False,
             unroll=False, decouple=False, strip_tr=False, strip_gates=False,
             use_bulk=False, hint_all=False, staggered=False):
    """Build + compile the per-core Bass program. T must be divisible by S.

    T_data: DRAM extent of xT (defaults to T). A smaller T with full T_data
    gives a calibration kernel with identical I/O but less compute.
    """
    assert T % S == 0
    if T_data is None:
        T_data = S if timing_mode else T
    R = F32R if use_f32r else F32
    nc = bacc.Bacc(None, target_bir_lowering=False)

    xT_d = nc.dram_tensor("xT", [KD, 128, T_data, BL], R, kind="ExternalInput")
    eps_d = nc.dram_tensor("eps", [BL, L], F32, kind="ExternalInput")
    Wx1_d = nc.dram_tensor("Wx1", [KD, 128, FH], R, kind="ExternalInput")
    Wh1_d = nc.dram_tensor("Wh1", [KH, 128, FH], R, kind="ExternalInput")
    Wx2_d = nc.dram_tensor("Wx2", [KH, 128, FH], R, kind="ExternalInput")
    Wh2_d = nc.dram_tensor("Wh2", [KH, 128, FH], R, kind="ExternalInput")
    b1_d = nc.dram_tensor("b1", [1, FH], R, kind="ExternalInput")
    b2_d = nc.dram_tensor("b2", [1, FH], R, kind="ExternalInput")
    wms_d = nc.dram_tensor("wms", [KH, 128, 2 * L], R, kind="ExternalInput")
    bms_d = nc.dram_tensor("bms", [1, 2 * L], R, kind="ExternalInput")
    ones_d = nc.dram_tensor("ones", [1, 128], R, kind="ExternalInput")
    zst_d = nc.dram_tensor("zstate", [128, KH * BL], R, kind="ExternalInput")
    ident_d = nc.dram_tensor("ident", [128, 128], F32, kind="ExternalInput")

    ret_d = nc.dram_tensor("retval", [BL, L], F32, kind="ExternalOutput")
    lp_d = nc.dram_tensor("loss_part", [BL, 1], F32, kind="ExternalOutput")
    c2_d = nc.dram_tensor("c2_out", [BL, H], F32, kind="ExternalOutput")
    h1_d = nc.dram_tensor("h1_out", [BL, H], F32, kind="ExternalOutput")
    c1_d = nc.dram_tensor("c1_out", [BL, H], F32, kind="ExternalOutput")

    with tile.TileContext(nc) as tc:
        with (
            tc.tile_pool(name="wpool", bufs=1) as wp,
            tc.tile_pool(name="xpool", bufs=2) as xp,
            tc.tile_pool(name="zxpool", bufs=1) as zxp,
            tc.tile_pool(name="state", bufs=1) as sp,
            tc.tile_pool(name="gates", bufs=1) as gp,
            tc.tile_pool(name="pz1", bufs=4, space="PSUM") as pz1,
            tc.tile_pool(name="pz2", bufs=4, space="PSUM") as pz2,
            tc.tile_pool(name="opool", bufs=1) as op,
        ):
            # ---- load weights & constants ----
            Wx1 = wp.tile([128, KD, FH], R, tag="Wx1")
            Wh1 = wp.tile([128, KH, FH], R, tag="Wh1")
            Wx2 = wp.tile([128, KH, FH], R, tag="Wx2")
            Wh2 = wp.tile([128, KH, FH], R, tag="Wh2")
            wms = wp.tile([128, KH, 2 * L], R, tag="wms")
            b1 = wp.tile([1, FH], R, tag="b1")
            b2 = wp.tile([1, FH], R, tag="b2")
            bms = wp.tile([1, 2 * L], R, tag="bms")
            ones = wp.tile([1, 128], R, tag="ones")
            ident = wp.tile([128, 128], F32, tag="ident")
            eps = wp.tile([BL, L], F32, tag="eps")

            nc.sync.dma_start(Wx1[:], Wx1_d.ap().rearrange("k p n -> p k n"))
            nc.sync.dma_start(Wh1[:], Wh1_d.ap().rearrange("k p n -> p k n"))
            nc.sync.dma_start(Wx2[:], Wx2_d.ap().rearrange("k p n -> p k n"))
            nc.sync.dma_start(Wh2[:], Wh2_d.ap().rearrange("k p n -> p k n"))
            nc.sync.dma_start(wms[:], wms_d.ap().rearrange("k p n -> p k n"))
            nc.sync.dma_start(b1[:], b1_d[:])
            nc.sync.dma_start(b2[:], b2_d[:])
            nc.sync.dma_start(bms[:], bms_d[:])
            nc.sync.dma_start(ones[:], ones_d[:])
            nc.sync.dma_start(ident[:], ident_d[:])
            nc.sync.dma_start(eps[:], eps_d[:])

            # ---- persistent state ----
            h1 = sp.tile([BL, H], F32, tag="h1")
            c1 = sp.tile([BL, H], F32, tag="c1")
            h2 = sp.tile([BL, H], F32, tag="h2")
            c2 = sp.tile([BL, H], F32, tag="c2")
            h1T = sp.tile([128, KH, BL], R, tag="h1T")
            h2T = sp.tile([128, KH, BL], R, tag="h2T")
            for t_ in (h1, c1, h2, c2):
                nc.vector.memset(t_[:], 0.0)
            nc.sync.dma_start(h1T.rearrange("p k b -> p (k b)"), zst_d[:])
            nc.sync.dma_start(h2T.rearrange("p k b -> p (k b)"), zst_d[:])
            if decouple:
                # timing experiment: MMs read frozen copies of hT so the
                # cross-step dependency chain is cut; instruction mix identical
                h1Tc = sp.tile([128, KH, BL], R, tag="h1Tc")
                h2Tc = sp.tile([128, KH, BL], R, tag="h2Tc")
                nc.sync.dma_start(h1Tc.rearrange("p k b -> p (k b)"), zst_d[:])
                nc.sync.dma_start(h2Tc.rearrange("p k b -> p (k b)"), zst_d[:])
            else:
                h1Tc, h2Tc = h1T, h2T

            def transpose_to(hsrc, hTdst, force=False):
                """hsrc [BL, H] -> hTdst [128, KH, BL] via PE transposes."""
                if strip_tr and not force:
                    return
                ps = pz1.tile([128, KH * BL], F32, tag="z1")
                for k in range(KH):
                    nc.tensor.transpose(
                        ps[:, k * BL : (k + 1) * BL],
                        hsrc[:, k * 128 : (k + 1) * 128],
                        ident[:BL, :BL],
                    )
                nc.vector.tensor_copy(hTdst.rearrange("p k b -> p (k b)"), ps[:])

            def lstm_step(*a, **kw):
                raise NotImplementedError

            # bias broadcast tiles [BL, FH] (one-time build via rank-1 MMs)
            b1b = wp.tile([BL, FH], F32, tag="b1b")
            b2b = wp.tile([BL, FH], F32, tag="b2b")
            for bsrc, bdst in ((b1, b1b), (b2, b2b)):
                for n in range(NCH):
                    nsl = slice(n * 512, (n + 1) * 512)
                    pt = pz2.tile([BL, 512], F32, tag="z2")
                    nc.tensor.matmul(
                        pt[:], ones[:1, :BL], bsrc[:1, nsl],
                        start=True, stop=True,
                    )
                    nc.vector.tensor_copy(bdst[:, nsl], pt[:])

            def gates_and_update(zps, adds, h, c, ztag):
                if strip_gates:
                    return
                g = {}
                for n in range(NCH):
                    nsl = slice(n * 512, (n + 1) * 512)
                    for a_ in adds:
                        nc.vector.tensor_add(zps[n][:], zps[n][:], a_[:, nsl])
                    gt = gp.tile([BL, 512], F32, tag=f"{ztag}g{n}")
                    fn = AF.Tanh if n == 2 else AF.Sigmoid
                    nc.scalar.activation(gt[:], zps[n][:], fn)
                    g[n] = gt
                i_, f_, g_, o_ = g[0], g[1], g[2], g[3]
                t1 = gp.tile([BL, H], F32, tag="t1")
                t2 = gp.tile([BL, H], F32, tag="t2")
                nc.vector.tensor_mul(t1[:], f_[:], c[:])
                nc.vector.tensor_mul(t2[:], i_[:], g_[:])
                nc.vector.tensor_add(c[:], t1[:], t2[:])
                th = gp.tile([BL, H], F32, tag="th")
                nc.scalar.activation(th[:], c[:], AF.Tanh)
                nc.vector.tensor_mul(h[:], o_[:], th[:])

            def emit_step(z1adds, carry, x_stat=None):
                """Emit one LSTM step; layer-2 gate/transpose of the previous
                step is deferred into this step's z1 window (carry holds the
                previous step's z2 PSUM chunks). z1adds: tensors DVE-added to
                the layer-1 PSUM. x_stat: per-step x stationary (non-bulk)."""
                z1ps = []
                for n in range(NCH):
                    nsl = slice(n * 512, (n + 1) * 512)
                    zp = pz1.tile([BL, 512], F32, tag="z1")
                    if x_stat is not None:
                        for k in range(KD):
                            nc.tensor.matmul(
                                zp[:], x_stat(k), Wx1[:, k, nsl],
                                start=(k == 0), stop=False,
                            )
                    for k in range(KH):
                        nc.tensor.matmul(
                            zp[:], h1Tc[:, k, :], Wh1[:, k, nsl],
                            start=(x_stat is None and k == 0),
                            stop=(k == KH - 1),
                        )
                    z1ps.append(zp)
                # finish the PREVIOUS step's layer 2 while z1 MMs run
                if carry is not None:
                    gates_and_update(carry, [b2b], h2, c2, "z2")
                    transpose_to(h2, h2T)
                # layer-2 z, recurrent half (h2T now current)
                z2ps = []
                for n in range(NCH):
                    nsl = slice(n * 512, (n + 1) * 512)
                    zp = pz2.tile([BL, 512], F32, tag="z2")
                    for k in range(KH):
                        nc.tensor.matmul(
                            zp[:], h2Tc[:, k, :], Wh2[:, k, nsl],
                            start=(k == 0), stop=False,
                        )
                    z2ps.append(zp)
                gates_and_update(z1ps, z1adds, h1, c1, "z1")
                transpose_to(h1, h1T)
                # layer-2 z, input half (needs fresh h1T)
                for n in range(NCH):
                    nsl = slice(n * 512, (n + 1) * 512)
                    for k in range(KH):
                        nc.tensor.matmul(
                            z2ps[n][:], h1Tc[:, k, :], Wx2[:, k, nsl],
                            start=False, stop=(k == KH - 1),
                        )
                return z2ps

            # ---- time loop ----
            def chunk_body(iv):
                xt = xp.tile([128, KD, S, BL], R, tag="xT")
                xsl = slice(0, S) if timing_mode else bass.ds(iv, S)
                nc.sync.dma_start(
                    xt[:],
                    xT_d.ap()[:, :, xsl, :].rearrange(
                        "k p t b -> p k t b"
                    ),
                )
                carry = None
                if use_bulk:
                    # bulk: zx = x@Wx1 for the whole chunk at M=128 (full PE
                    # util). NOTE: measured slower than per-step x MMs — the
                    # PSUM->SBUF copies delay gate activations. Kept for
                    # experiments.
                    JT = S // 4
                    zxa = zxp.tile([128, JT, FH], F32, tag="zx")
                    for j in range(JT):
                        for n in range(NCH):
                            nsl = slice(n * 512, (n + 1) * 512)
                            bp = pz1.tile([128, 512], F32, tag="z1")
                            for k in range(KD):
                                nc.tensor.matmul(
                                    bp[:],
                                    xt[:, k, 4 * j : 4 * j + 4, :].rearrange(
                                        "p t b -> p (t b)"
                                    ),
                                    Wx1[:, k, nsl],
                                    start=(k == 0), stop=(k == KD - 1),
                                )
                            nc.scalar.copy(zxa[:, j, nsl], bp[:])
                    for s in range(S):
                        zxs = zxa[32 * (s % 4) : 32 * (s % 4) + 32, s // 4, :]
                        carry = emit_step([b1b, zxs], carry)
                else:
                    for s in range(S):
                        carry = emit_step(
                            [b1b], carry, x_stat=lambda k, s=s: xt[:, k, s, :]
                        )
                # drain the last step's layer 2 before the back-edge barrier
                gates_and_update(carry, [b2b], h2, c2, "z2")
                transpose_to(h2, h2T)

            if unroll:
                assert timing_mode or T == S
                for it in range(T // S):
                    chunk_body(it * S)
            elif T // S > 1:
                hints = (
                    (mybir.EngineType.PE, mybir.EngineType.DVE,
                     mybir.EngineType.Activation, mybir.EngineType.SP)
                    if hint_all else (mybir.EngineType.PE,)
                )
                with tc.For_i(0, T, S, hint_engines=hints,
                              staggered_reset=staggered) as iv:
                    chunk_body(iv)
            else:
                chunk_body(0)

            # ---- latent head ----
            c2T = sp.tile([128, KH, BL], R, tag="c2T")
            transpose_to(c2, c2T, force=True)
            pms = pz1.tile([BL, 512], F32, tag="z1")
            for k in range(KH):
                nc.tensor.matmul(
                    pms[:, : 2 * L], c2T[:, k, :], wms[:, k, :],
                    start=(k == 0), stop=False,
                )
            nc.tensor.matmul(
                pms[:, : 2 * L], ones[:1, :BL], bms[:1, :],
                start=False, stop=True,
            )
            mean = pms[:, 0:L]
            sigma = pms[:, L : 2 * L]

            ehalf = op.tile([BL, L], F32, tag="ehalf")
            nc.scalar.activation(ehalf[:], sigma, AF.Exp, scale=0.5)
            rv = op.tile([BL, L], F32, tag="rv")
            nc.vector.tensor_mul(rv[:], ehalf[:], eps[:])
            nc.vector.tensor_add(rv[:], rv[:], mean)
            nc.sync.dma_start(ret_d[:], rv[:])

            # loss partials: u = sigma - mean^2 - exp(sigma), summed over L
            sq = op.tile([BL, L], F32, tag="sq")
            nc.scalar.activation(sq[:], mean, AF.Square)
            ex = op.tile([BL, L], F32, tag="ex")
            nc.scalar.activation(ex[:], sigma, AF.Exp)
            u = op.tile([BL, L], F32, tag="u")
            nc.vector.tensor_sub(u[:], sigma, sq[:])
            nc.vector.tensor_sub(u[:], u[:], ex[:])
            lp = op.tile([BL, 1], F32, tag="lp")
            nc.vector.reduce_sum(lp[:], u[:], axis=mybir.AxisListType.X)
            nc.sync.dma_start(lp_d[:], lp[:])

            co = op.tile([BL, H], F32, tag="co")
            nc.vector.tensor_copy(co[:], c2[:])
            nc.sync.dma_start(c2_d[:], co[:])
            ho = op.tile([BL, H], F32, tag="co")
            nc.vector.tensor_copy(ho[:], h1[:])
            nc.sync.dma_start(h1_d[:], ho[:])
            c1o = op.tile([BL, H], F32, tag="co")
            nc.vector.tensor_copy(c1o[:], c1[:])
            nc.sync.dma_start(c1_d[:], c1o[:])

    nc.compile()
    return nc


def prep_core_inputs(inputs, eps, Wx1, Wh1, b1, Wx2, Wh2, b2,
                     w_mean, b_mean, w_sigma, b_sigma, T=T_FULL):
    """Build the per-core in_maps list (host-side shard + relayout)."""
    f = np.float32
    shared = {
        "Wx1": np.ascontiguousarray(Wx1.reshape(KD, 128, FH), f),
        "Wh1": np.ascontiguousarray(Wh1.reshape(KH, 128, FH), f),
        "Wx2": np.ascontiguousarray(Wx2.reshape(KH, 128, FH), f),
        "Wh2": np.ascontiguousarray(Wh2.reshape(KH, 128, FH), f),
        "b1": np.ascontiguousarray(b1.reshape(1, FH), f),
        "b2": np.ascontiguousarray(b2.reshape(1, FH), f),
        "wms": np.ascontiguousarray(
            np.concatenate([w_mean, w_sigma], axis=1).reshape(KH, 128, 2 * L), f
        ),
        "bms": np.ascontiguousarray(
            np.concatenate([b_mean, b_sigma]).reshape(1, 2 * L), f
        ),
        "ones": np.ones((1, 128), f),
        "zstate": np.zeros((128, KH * BL), f),
        "ident": np.eye(128, dtype=f),
    }
    in_maps = []
    for c in range(NCORES):
        sl = slice(c * BL, (c + 1) * BL)
        xc = np.asarray(inputs[sl, :T], f)  # [BL, T, D]
        xT = np.ascontiguousarray(xc.transpose(2, 1, 0)).reshape(KD, 128, T, BL)
        m = dict(shared)
        m["xT"] = xT
        m["eps"] = np.ascontiguousarray(eps[sl], f)
        in_maps.append(m)
    return in_maps


_NC_CACHE = {}


def _get_nc(T=T_FULL, S=16):
    key = (T, S)
    if key not in _NC_CACHE:
        _NC_CACHE[key] = build_nc(T=T, S=S)
    return _NC_CACHE[key]


def kernel(inputs, eps, Wx1, Wh1, b1, Wx2, Wh2, b2,
           w_mean, b_mean, w_sigma, b_sigma):
    T = inputs.shape[1]
    nc = _get_nc(T=T, S=16)
    in_maps = prep_core_inputs(
        inputs, eps, Wx1, Wh1, b1, Wx2, Wh2, b2,
        w_mean, b_mean, w_sigma, b_sigma, T=T,
    )
    res = run_bass_kernel_spmd(nc, in_maps, core_ids=list(range(NCORES)))
    retval = np.concatenate([r["retval"] for r in res.results], axis=0)
    total = float(sum(r["loss_part"].sum() for r in res.results))
    loss = np.float32(-0.5 * (1.0 + total / (B * L)))
    return retval, loss


# revision 23
# speedup vs baseline: 1.2777x; 1.1562x over previous
"""Trainium2 Bass kernel for nn_Encoder_5171140624511.

2-layer LSTM encoder (B=256, T=1024, D_IN=256, H=512) + VAE latent head.
Sharding: data-parallel over batch across 8 NeuronCores (32 samples/core),
LSTM/projection weights replicated.

Layout strategy per core:
  - state h, c in [batch=32 partitions, H free]; z = x@Wx + h@Wh + b computed
    as [32, 4H] in PSUM with the *weights as the moving operand* (float32r,
    1 cycle/row at N=512) and hT/xT as the 128x32 stationary tiles.
  - h is re-transposed each step via 4 PE transposes into hT [128, 4, 32].
  - x is pre-transposed on the host to xT [2, 128, T, 32] so per-step
    stationary x tiles DMA straight in.
  - latent head (mean/sigma projections, reparameterization, loss partials)
    computed on device; host only concatenates shards and finishes the
    scalar loss reduction.
"""

import os
import sys

sys.path.insert(0, "/opt/trn_rl_repo")

import numpy as np

import concourse.bass as bass
import concourse.tile as tile
from concourse import bacc, mybir
from concourse.bass_utils import run_bass_kernel_spmd

B, T_FULL, D, H, L = 256, 1024, 256, 512, 128
FH = 4 * H  # 2048
NCORES = 8
BL = B // NCORES  # 32
KH = H // 128  # 4 k-tiles for H
KD = D // 128  # 2 k-tiles for D_IN
NCH = FH // 512  # 4 n-chunks of 512

F32 = mybir.dt.float32
F32R = mybir.dt.float32r
AF = mybir.ActivationFunctionType


def _r(ap):
    """View an AP as float32r for fast fp32 matmul."""
    return ap.bitcast(F32R)


def build_nc(T=T_FULL, S=8, use_f32r=True, T_data=None, timing_mode=False,
             unroll=False, decouple=False, strip_tr=False, strip_gates=False,
             use_bulk=False, hint_all=False, staggered=False):
    """Build + compile the per-core Bass program. T must be divisible by S.

    T_data: DRAM extent of xT (defaults to T). A smaller T with full T_data
    gives a calibration kernel with identical I/O but less compute.
    """
    assert T % S == 0
    if T_data is None:
        T_data = S if timing_mode else T
    R = F32R if use_f32r else F32
    nc = bacc.Bacc(None, target_bir_lowering=False)

    xT_d = nc.dram_tensor("xT", [KD, 128, T_data, BL], R, kind="ExternalInput")
    eps_d = nc.dram_tensor("eps", [BL, L], F32, kind="ExternalInput")
    Wx1_d = nc.dram_tensor("Wx1", [KD, 128, FH], R, kind="ExternalInput")
    Wh1_d = nc.dram_tensor("Wh1", [KH, 128, FH], R, kind="ExternalInput")
    Wx2_d = nc.dram_tensor("Wx2", [KH, 128, FH], R, kind="ExternalInput")
    Wh2_d = nc.dram_tensor("Wh2", [KH, 128, FH], R, kind="ExternalInput")
    b1_d = nc.dram_tensor("b1", [1, FH], R, kind="ExternalInput")
    b2_d = nc.dram_tensor("b2", [1, FH], R, kind="ExternalInput")
    wms_d = nc.dram_tensor("wms", [KH, 128, 2 * L], R, kind="ExternalInput")
    bms_d = nc.dram_tensor("bms", [1, 2 * L], R, kind="ExternalInput")
    ones_d = nc.dram_tensor("ones", [1, 128], R, kind="ExternalInput")
    zst_d = nc.dram_tensor("zstate", [128, KH * BL], R, kind="ExternalInput")
    ident_d = nc.dram_tensor("ident", [128, 128], F32, kind="ExternalInput")

    ret_d = nc.dram_tensor("retval", [BL, L], F32, kind="ExternalOutput")
    lp_d = nc.dram_tensor("loss_part", [BL, 1], F32, kind="ExternalOutput")
    c2_d = nc.dram_tensor("c2_out", [BL, H], F32, kind="ExternalOutput")
    h1_d = nc.dram_tensor("h1_out", [BL, H], F32, kind="ExternalOutput")
    c1_d = nc.dram_tensor("c1_out", [BL, H], F32, kind="ExternalOutput")

    with tile.TileContext(nc) as tc:
        with (
            tc.tile_pool(name="wpool", bufs=1) as wp,
            tc.tile_pool(name="xpool", bufs=2) as xp,
            tc.tile_pool(name="zxpool", bufs=1) as zxp,
            tc.tile_pool(name="state", bufs=1) as sp,
            tc.tile_pool(name="gates", bufs=1) as gp,
            tc.tile_pool(name="pz1", bufs=4, space="PSUM") as pz1,
            tc.tile_pool(name="pz2", bufs=4, space="PSUM") as pz2,
            tc.tile_pool(name="opool", bufs=1) as op,
        ):
            # ---- load weights & constants ----
            Wx1 = wp.tile([128, KD, FH], R, tag="Wx1")
            Wh1 = wp.tile([128, KH, FH], R, tag="Wh1")
            Wx2 = wp.tile([128, KH, FH], R, tag="Wx2")
            Wh2 = wp.tile([128, KH, FH], R, tag="Wh2")
            wms = wp.tile([128, KH, 2 * L], R, tag="wms")
            b1 = wp.tile([1, FH], R, tag="b1")
            b2 = wp.tile([1, FH], R, tag="b2")
            bms = wp.tile([1, 2 * L], R, tag="bms")
            ones = wp.tile([1, 128], R, tag="ones")
            ident = wp.tile([128, 128], F32, tag="ident")
            eps = wp.tile([BL, L], F32, tag="eps")

            nc.sync.dma_start(Wx1[:], Wx1_d.ap().rearrange("k p n -> p k n"))
            nc.sync.dma_start(Wh1[:], Wh1_d.ap().rearrange("k p n -> p k n"))
            nc.sync.dma_start(Wx2[:], Wx2_d.ap().rearrange("k p n -> p k n"))
            nc.sync.dma_start(Wh2[:], Wh2_d.ap().rearrange("k p n -> p k n"))
            nc.sync.dma_start(wms[:], wms_d.ap().rearrange("k p n -> p k n"))
            nc.sync.dma_start(b1[:], b1_d[:])
            nc.sync.dma_start(b2[:], b2_d[:])
            nc.sync.dma_start(bms[:], bms_d[:])
            nc.sync.dma_start(ones[:], ones_d[:])
            nc.sync.dma_start(ident[:], ident_d[:])
            nc.sync.dma_start(eps[:], eps_d[:])

            # ---- persistent state ----
            h1 = sp.tile([BL, H], F32, tag="h1")
            c1 = sp.tile([BL, H], F32, tag="c1")
            h2 = sp.tile([BL, H], F32, tag="h2")
            c2 = sp.tile([BL, H], F32, tag="c2")
            h1T = sp.tile([128, KH, BL], R, tag="h1T")
            h2T = sp.tile([128, KH, BL], R, tag="h2T")
            for t_ in (h1, c1, h2, c2):
                nc.vector.memset(t_[:], 0.0)
            nc.sync.dma_start(h1T.rearrange("p k b -> p (k b)"), zst_d[:])
            nc.sync.dma_start(h2T.rearrange("p k b -> p (k b)"), zst_d[:])
            if decouple:
                # timing experiment: MMs read frozen copies of hT so the
                # cross-step dependency chain is cut; instruction mix identical
                h1Tc = sp.tile([128, KH, BL], R, tag="h1Tc")
                h2Tc = sp.tile([128, KH, BL], R, tag="h2Tc")
                nc.sync.dma_start(h1Tc.rearrange("p k b -> p (k b)"), zst_d[:])
                nc.sync.dma_start(h2Tc.rearrange("p k b -> p (k b)"), zst_d[:])
            else:
                h1Tc, h2Tc = h1T, h2T

            def transpose_to(hsrc, hTdst, force=False):
                """hsrc [BL, H] -> hTdst [128, KH, BL] via PE transposes."""
                if strip_tr and not force:
                    return
                ps = pz1.tile([128, KH * BL], F32, tag="z1")
                for k in range(KH):
                    nc.tensor.transpose(
                        ps[:, k * BL : (k + 1) * BL],
                        hsrc[:, k * 128 : (k + 1) * 128],
                        ident[:BL, :BL],
                    )
                nc.vector.tensor_copy(hTdst.rearrange("p k b -> p (k b)"), ps[:])

            def lstm_step(*a, **kw):
                raise NotImplementedError

            # bias broadcast tiles [BL, FH] (one-time build via rank-1 MMs)
            b1b = wp.tile([BL, FH], F32, tag="b1b")
            b2b = wp.tile([BL, FH], F32, tag="b2b")
            for bsrc, bdst in ((b1, b1b), (b2, b2b)):
                for n in range(NCH):
                    nsl = slice(n * 512, (n + 1) * 512)
                    pt = pz2.tile([BL, 512], F32, tag="z2")
                    nc.tensor.matmul(
                        pt[:], ones[:1, :BL], bsrc[:1, nsl],
                        start=True, stop=True,
                    )
                    nc.vector.tensor_copy(bdst[:, nsl], pt[:])

            def gates_and_update(zps, adds, h, c, ztag):
                if strip_gates:
                    return
                g = {}
                for n in range(NCH):
                    nsl = slice(n * 512, (n + 1) * 512)
                    for a_ in adds:
                        nc.vector.tensor_add(zps[n][:], zps[n][:], a_[:, nsl])
                    gt = gp.tile([BL, 512], F32, tag=f"{ztag}g{n}")
                    fn = AF.Tanh if n == 2 else AF.Sigmoid
                    nc.scalar.activation(gt[:], zps[n][:], fn)
                    g[n] = gt
                i_, f_, g_, o_ = g[0], g[1], g[2], g[3]
                t1 = gp.tile([BL, H], F32, tag="t1")
                t2 = gp.tile([BL, H], F32, tag="t2")
                nc.vector.tensor_mul(t1[:], f_[:], c[:])
                nc.vector.tensor_mul(t2[:], i_[:], g_[:])
                nc.vector.tensor_add(c[:], t1[:], t2[:])
                th = gp.tile([BL, H], F32, tag="th")
                nc.scalar.activation(th[:], c[:], AF.Tanh)
                nc.vector.tensor_mul(h[:], o_[:], th[:])

            def emit_step(z1adds, carry, x_stat=None):
                """Emit one LSTM step; layer-2 gate/transpose of the previous
                step is deferred into this step's z1 window (carry holds the
                previous step's z2 PSUM chunks). z1adds: tensors DVE-added to
                the layer-1 PSUM. x_stat: per-step x stationary (non-bulk)."""
                z1ps = []
                for n in range(NCH):
                    nsl = slice(n * 512, (n + 1) * 512)
                    zp = pz1.tile([BL, 512], F32, tag="z1")
                    if x_stat is not None:
                        for k in range(KD):
                            nc.tensor.matmul(
                                zp[:], x_stat(k), Wx1[:, k, nsl],
                                start=(k == 0), stop=False,
                            )
                    for k in range(KH):
                        nc.tensor.matmul(
                            zp[:], h1Tc[:, k, :], Wh1[:, k, nsl],
                            start=(x_stat is None and k == 0),
                            stop=(k == KH - 1),
                        )
                    z1ps.append(zp)
                # finish the PREVIOUS step's layer 2 while z1 MMs run
                if carry is not None:
                    gates_and_update(carry, [b2b], h2, c2, "z2")
                    transpose_to(h2, h2T)
                # layer-2 z, recurrent half (h2T now current)
                z2ps = []
                for n in range(NCH):
                    nsl = slice(n * 512, (n + 1) * 512)
                    zp = pz2.tile([BL, 512], F32, tag="z2")
                    for k in range(KH):
                        nc.tensor.matmul(
                            zp[:], h2Tc[:, k, :], Wh2[:, k, nsl],
                            start=(k == 0), stop=False,
                        )
                    z2ps.append(zp)
                gates_and_update(z1ps, z1adds, h1, c1, "z1")
                transpose_to(h1, h1T)
                # layer-2 z, input half (needs fresh h1T)
                for n in range(NCH):
                    nsl = slice(n * 512, (n + 1) * 512)
                    for k in range(KH):
                        nc.tensor.matmul(
                            z2ps[n][:], h1Tc[:, k, :], Wx2[:, k, nsl],
                            start=False, stop=(k == KH - 1),
                        )
                return z2ps

            # ---- time loop ----
            def chunk_body(iv):
                xt = xp.tile([128, KD, S, BL], R, tag="xT")
                xsl = slice(0, S) if timing_mode else bass.ds(iv, S)
                nc.sync.dma_start(
                    xt[:],
                    xT_d.ap()[:, :, xsl, :].rearrange(
                        "k p t b -> p k t b"
                    ),
                )
                carry = None
                if use_bulk:
                    # bulk x@Wx1 per 4-step rows-tile at M=128 (full PE util),
                    # emitted just-in-time before its 4 consumer steps; the
                    # PSUM->SBUF copies go to DVE so ACT gates aren't blocked.
                    for j in range(S // 4):
                        zxa = zxp.tile([128, FH], F32, tag="zx")
                        for n in range(NCH):
                            nsl = slice(n * 512, (n + 1) * 512)
                            bp = pz1.tile([128, 512], F32, tag="z1")
                            for k in range(KD):
                                nc.tensor.matmul(
                                    bp[:],
                                    xt[:, k, 4 * j : 4 * j + 4, :].rearrange(
                                        "p t b -> p (t b)"
                                    ),
                                    Wx1[:, k, nsl],
                                    start=(k == 0), stop=(k == KD - 1),
                                )
                            nc.vector.tensor_copy(zxa[:, nsl], bp[:])
                        for s in range(4 * j, 4 * j + 4):
                            zxs = zxa[32 * (s % 4) : 32 * (s % 4) + 32, :]
                            carry = emit_step([b1b, zxs], carry)
                else:
                    for s in range(S):
                        carry = emit_step(
                            [b1b], carry, x_stat=lambda k, s=s: xt[:, k, s, :]
                        )
                # drain the last step's layer 2 before the back-edge barrier
                gates_and_update(carry, [b2b], h2, c2, "z2")
                transpose_to(h2, h2T)

            if unroll:
                assert timing_mode or T == S
                for it in range(T // S):
                    chunk_body(it * S)
            elif T // S > 1:
                hints = (
                    (mybir.EngineType.PE, mybir.EngineType.DVE,
                     mybir.EngineType.Activation, mybir.EngineType.SP)
                    if hint_all else (mybir.EngineType.PE,)
                )
                with tc.For_i(0, T, S, hint_engines=hints,
                              staggered_reset=staggered) as iv:
                    chunk_body(iv)
            else:
                chunk_body(0)

            # ---- latent head ----
            c2T = sp.tile([128, KH, BL], R, tag="c2T")
            transpose_to(c2, c2T, force=True)
            pms = pz1.tile([BL, 512], F32, tag="z1")
            for k in range(KH):
                nc.tensor.matmul(
                    pms[:, : 2 * L], c2T[:, k, :], wms[:, k, :],
                    start=(k == 0), stop=False,
                )
            nc.tensor.matmul(
                pms[:, : 2 * L], ones[:1, :BL], bms[:1, :],
                start=False, stop=True,
            )
            mean = pms[:, 0:L]
            sigma = pms[:, L : 2 * L]

            ehalf = op.tile([BL, L], F32, tag="ehalf")
            nc.scalar.activation(ehalf[:], sigma, AF.Exp, scale=0.5)
            rv = op.tile([BL, L], F32, tag="rv")
            nc.vector.tensor_mul(rv[:], ehalf[:], eps[:])
            nc.vector.tensor_add(rv[:], rv[:], mean)
            nc.sync.dma_start(ret_d[:], rv[:])

            # loss partials: u = sigma - mean^2 - exp(sigma), summed over L
            sq = op.tile([BL, L], F32, tag="sq")
            nc.scalar.activation(sq[:], mean, AF.Square)
            ex = op.tile([BL, L], F32, tag="ex")
            nc.scalar.activation(ex[:], sigma, AF.Exp)
            u = op.tile([BL, L], F32, tag="u")
            nc.vector.tensor_sub(u[:], sigma, sq[:])
            nc.vector.tensor_sub(u[:], u[:], ex[:])
            lp = op.tile([BL, 1], F32, tag="lp")
            nc.vector.reduce_sum(lp[:], u[:], axis=mybir.AxisListType.X)
            nc.sync.dma_start(lp_d[:], lp[:])

            co = op.tile([BL, H], F32, tag="co")
            nc.vector.tensor_copy(co[:], c2[:])
            nc.sync.dma_start(c2_d[:], co[:])
            ho = op.tile([BL, H], F32, tag="co")
            nc.vector.tensor_copy(ho[:], h1[:])
            nc.sync.dma_start(h1_d[:], ho[:])
            c1o = op.tile([BL, H], F32, tag="co")
            nc.vector.tensor_copy(c1o[:], c1[:])
            nc.sync.dma_start(c1_d[:], c1o[:])

    nc.compile()
    return nc


def prep_core_inputs(inputs, eps, Wx1, Wh1, b1, Wx2, Wh2, b2,
                     w_mean, b_mean, w_sigma, b_sigma, T=T_FULL):
    """Build the per-core in_maps list (host-side shard + relayout)."""
    f = np.float32
    shared = {
        "Wx1": np.ascontiguousarray(Wx1.reshape(KD, 128, FH), f),
        "Wh1": np.ascontiguousarray(Wh1.reshape(KH, 128, FH), f),
        "Wx2": np.ascontiguousarray(Wx2.reshape(KH, 128, FH), f),
        "Wh2": np.ascontiguousarray(Wh2.reshape(KH, 128, FH), f),
        "b1": np.ascontiguousarray(b1.reshape(1, FH), f),
        "b2": np.ascontiguousarray(b2.reshape(1, FH), f),
        "wms": np.ascontiguousarray(
            np.concatenate([w_mean, w_sigma], axis=1).reshape(KH, 128, 2 * L), f
        ),
        "bms": np.ascontiguousarray(
            np.concatenate([b_mean, b_sigma]).reshape(1, 2 * L), f
        ),
        "ones": np.ones((1, 128), f),
        "zstate": np.zeros((128, KH * BL), f),
        "ident": np.eye(128, dtype=f),
    }
    in_maps = []
    for c in range(NCORES):
        sl = slice(c * BL, (c + 1) * BL)
        xc = np.asarray(inputs[sl, :T], f)  # [BL, T, D]
        xT = np.ascontiguousarray(xc.transpose(2, 1, 0)).reshape(KD, 128, T, BL)
        m = dict(shared)
        m["xT"] = xT
        m["eps"] = np.ascontiguousarray(eps[sl], f)
        in_maps.append(m)
    return in_maps


_NC_CACHE = {}


def _get_nc(T=T_FULL, S=16):
    key = (T, S)
    if key not in _NC_CACHE:
        _NC_CACHE[key] = build_nc(T=T, S=S)
    return _NC_CACHE[key]


def kernel(inputs, eps, Wx1, Wh1, b1, Wx2, Wh2, b2,
           w_mean, b_mean, w_sigma, b_sigma):
    T = inputs.shape[1]
    nc = _get_nc(T=T, S=16)
    in_maps = prep_core_inputs(
        inputs, eps, Wx1, Wh1, b1, Wx2, Wh2, b2,
        w_mean, b_mean, w_sigma, b_sigma, T=T,
    )
    res = run_bass_kernel_spmd(nc, in_maps, core_ids=list(range(NCORES)))
    retval = np.concatenate([r["retval"] for r in res.results], axis=0)
    total = float(sum(r["loss_part"].sum() for r in res.results))
    loss = np.float32(-0.5 * (1.0 + total / (B * L)))
    return retval, loss
